# revision 1
# baseline (speedup 1.0000x reference)
"""Trainium2 Bass kernel for TernaryLinear: y[b,m,n] = sum_k x[b,m,k] * w[k,n].

Shapes: x (4, 2048, 4096) fp32, w (4096, 4096) ternary fp32 -> y (4, 2048, 4096).

Strategy: flatten x to 8192 rows, row-shard across 8 NeuronCores (1024 rows
each), replicate w. Per core: keep x^T resident in SBUF as 32 k-tiles of
[128k x 1024m] fp16 (the stationary matmul operand; fp16 weights get the
fast-weight-load path so the per-matmul weight load hides under the previous
matmul), stream w as [128k x 512n] fp16 tiles (ternary {-1,0,1} is exact in
fp16), accumulate over K into 8 PSUM banks (one per 128-row m-tile) in fp32,
evict PSUM->SBUF alternating between the vector and scalar engines, DMA
results out. No cross-core communication; host concatenates the row shards.
"""

import sys

for _p in ("/opt/trn_rl_repo", "/opt/pypackages"):
    if _p not in sys.path:
        sys.path.append(_p)

import numpy as np

import concourse.bass as bass
import concourse.bacc as bacc
import concourse.mybir as mybir
import concourse.tile as tile
from concourse.bass_utils import run_bass_kernel_spmd

P = 128
NCORES = 8
B, M, K, N = 4, 2048, 4096, 4096
R = B * M            # 8192 rows total
MR = R // NCORES     # 1024 rows per core
KT = K // P          # 32 k-tiles
MT = MR // P         # 8 m-tiles per core
NCH = 512            # moving free dim per matmul (one PSUM bank of fp32)
NCHUNKS = N // NCH   # 8
F32 = mybir.dt.float32
F16 = mybir.dt.float16

_PROGRAM = None


def _build_program():
    nc = bacc.Bacc(
        "TRN2",
        target_bir_lowering=False,
        debug=False,
        num_devices=NCORES,
    )
    xt = nc.dram_tensor("xt", [P, KT, MT, P], F16, kind="ExternalInput").ap()
    w = nc.dram_tensor("w", [NCHUNKS, KT, P, NCH], F16, kind="ExternalInput").ap()
    y = nc.dram_tensor("y", [MT, P, N], F32, kind="ExternalOutput").ap()

    with tile.TileContext(nc) as tc:
        with (
            tc.tile_pool(name="xres", bufs=1) as xpool,
            tc.tile_pool(name="wstream", bufs=10) as wpool,
            tc.tile_pool(name="outstage", bufs=8) as opool,
            tc.tile_pool(name="acc", bufs=8, space="PSUM") as ppool,
        ):
            # x^T resident: one tile per k-tile, [128 kp, MT, 128 m]. The
            # loads are interleaved with the first n-chunk's w stream (same
            # DMA issue queue) so the PE starts after one x slice + one w
            # tile instead of after the whole x preload.
            xtiles = [None] * KT

            def evict(nch, mt, ps):
                ot = opool.tile([P, NCH], F32, tag="o", name=f"o{nch}_{mt}")
                if mt % 2 == 0:
                    nc.vector.tensor_copy(ot[:], ps[:])
                else:
                    nc.scalar.copy(ot[:], ps[:])
                # Alternate output DMAs across two HWDGE queues so they don't
                # serialize behind each other (or the w-input stream).
                dma_eng = nc.scalar if mt % 2 == 0 else nc.sync
                dma_eng.dma_start(out=y[mt, :, bass.ts(nch, NCH)], in_=ot[:])

            for nch in range(NCHUNKS - 1):
                psums = [
                    ppool.tile([P, NCH], F32, tag="acc", name=f"ps{nch}_{mt}")
                    for mt in range(MT)
                ]
                for kt in range(KT):
                    if nch == 0:
                        xtile = xpool.tile(
                            [P, MT, P], F16, tag=f"x{kt}", name=f"x{kt}"
                        )
                        nc.sync.dma_start(out=xtile[:], in_=xt[:, kt])
                        xtiles[kt] = xtile
                    wt = wpool.tile([P, NCH], F16, tag="w", name=f"w{nch}_{kt}")
                    # During n-chunk 0 the sync queue is busy with the x
                    # preload; issue w loads on the scalar queue in parallel.
                    (nc.scalar if nch == 0 else nc.sync).dma_start(
                        out=wt[:], in_=w[nch, kt]
                    )
                    for mt in range(MT):
                        nc.tensor.matmul(
                            out=psums[mt][:],
                            lhsT=xtiles[kt][:, mt, :],
                            rhs=wt[:],
                            start=(kt == 0),
                            stop=(kt == KT - 1),
                        )
                for mt in range(MT):
                    evict(nch, mt, psums[mt])

            # Last n-chunk: mt-outer / kt-inner so each m-tile's accumulation
            # finishes early and its eviction + output DMA overlap the
            # remaining matmul stream; only the last m-tile drains after the
            # final matmul. Needs all 32 w tiles live at once (own slots).
            nch = NCHUNKS - 1
            wlast = []
            for kt in range(KT):
                wt = wpool.tile(
                    [P, NCH], F16, tag=f"wl{kt}", name=f"wl{kt}", bufs=1
                )
                nc.sync.dma_start(out=wt[:], in_=w[nch, kt])
                wlast.append(wt)
            for mt in range(MT):
                ps = ppool.tile([P, NCH], F32, tag="acc", name=f"psL_{mt}")
                for kt in range(KT):
                    nc.tensor.matmul(
                        out=ps[:],
                        lhsT=xtiles[kt][:, mt, :],
                        rhs=wlast[kt][:],
                        start=(kt == 0),
                        stop=(kt == KT - 1),
                    )
                evict(nch, mt, ps)
    nc.compile()
    return nc


def _get_program():
    global _PROGRAM
    if _PROGRAM is None:
        _PROGRAM = _build_program()
    return _PROGRAM


def _prepare_in_maps(x: np.ndarray, w: np.ndarray):
    x = np.ascontiguousarray(x, dtype=np.float32)
    w = np.ascontiguousarray(w, dtype=np.float32)
    # x rows -> [core, mt, mp, kt, kp] -> [core, kp, kt, mt, mp], fp16
    xr = x.reshape(NCORES, MT, P, KT, P)
    xt_all = np.ascontiguousarray(
        xr.transpose(0, 4, 3, 1, 2).astype(np.float16)
    )
    # w [kt, kp, nch, nn] -> [nch, kt, kp, nn], fp16 (exact for ternary)
    wr = np.ascontiguousarray(
        w.reshape(KT, P, NCHUNKS, NCH).transpose(2, 0, 1, 3).astype(np.float16)
    )
    return [{"xt": xt_all[c], "w": wr} for c in range(NCORES)]


def _gather_output(results):
    y = np.stack([np.asarray(r["y"]) for r in results])  # [core, MT, P, N]
    return y.reshape(B, M, N)


def run(x: np.ndarray, w: np.ndarray, trace: bool = False):
    """Returns (y, BassKernelResults)."""
    nc = _get_program()
    in_maps = _prepare_in_maps(x, w)
    res = run_bass_kernel_spmd(
        nc, in_maps, core_ids=list(range(NCORES)), trace=trace
    )
    return _gather_output(res.results), res


def kernel(x: np.ndarray, w: np.ndarray) -> np.ndarray:
    y, _ = run(x, w, trace=False)
    return y



# revision 2
# speedup vs baseline: 1.0056x; 1.0056x over previous
"""Trainium2 Bass kernel for TernaryLinear: y[b,m,n] = sum_k x[b,m,k] * w[k,n].

Shapes: x (4, 2048, 4096) fp32, w (4096, 4096) ternary fp32 -> y (4, 2048, 4096).

Strategy: flatten x to 8192 rows, row-shard across 8 NeuronCores (1024 rows
each), replicate w. All matmuls run in fp8e4 (e4m3, bias 8) with the
MatmulPerfMode.DoubleRow perf mode: each matmul instruction contracts K=256
(two 128-row k-subtiles packed in the middle AP dim) at 0.5 PE cycles per
output column -- 4x the fp16 rate. The ternary weight {-1,0,1} is exact in
fp8. x is split into x_hi = e4m3(x) plus a residual x_lo = e4m3(x - x_hi);
both streams multiply the SAME w tiles and accumulate into the same PSUM
group, so the correction costs G/16 extra matmuls and no extra w traffic.
With G=16 the end-to-end rel err is ~1.8e-3 (bf16 output); pure fp8 would be
2.7e-2.

Per core: x^T hi/lo resident in SBUF (2 KB/partition per double-k-tile),
w streamed as [128, 2, 512] fp8 tiles, 8 PSUM banks (one per 128-row m-tile),
PSUM evicted as bf16 (host casts back to fp32). No cross-core communication.
"""

import sys

for _p in ("/opt/trn_rl_repo", "/opt/pypackages"):
    if _p not in sys.path:
        sys.path.append(_p)

import ml_dtypes
import numpy as np

import concourse.bass as bass
import concourse.bacc as bacc
import concourse.mybir as mybir
import concourse.tile as tile
from concourse.bass_utils import run_bass_kernel_spmd

P = 128
NCORES = 8
B, M, K, N = 4, 2048, 4096, 4096
R = B * M            # 8192 rows total
MR = R // NCORES     # 1024 rows per core
DKT = K // (2 * P)   # 16 double-k-tiles (256 contraction each)
MT = MR // P         # 8 m-tiles per core
NCH = 512            # moving free dim per matmul (one PSUM bank of fp32)
NCHUNKS = N // NCH   # 8
WG = 2               # double-k-tiles per w DMA
NWG = DKT // WG      # 8 w DMAs per n-chunk
G = 16               # double-k-tiles that get the x_lo residual correction
F32 = mybir.dt.float32
BF16 = mybir.dt.bfloat16
F8 = mybir.dt.float8e4
E4M3 = ml_dtypes.float8_e4m3
DR = mybir.MatmulPerfMode.DoubleRow

_PROGRAM = None


def _build_program():
    nc = bacc.Bacc(
        "TRN2",
        target_bir_lowering=False,
        debug=False,
        num_devices=NCORES,
    )
    xt = nc.dram_tensor("xt", [P, DKT, 2, MT, P], F8, kind="ExternalInput").ap()
    xl = nc.dram_tensor("xl", [P, G, 2, MT, P], F8, kind="ExternalInput").ap()
    w = nc.dram_tensor("w", [NCHUNKS, NWG, P, WG, 2, NCH], F8, kind="ExternalInput").ap()
    y = nc.dram_tensor("y", [MT, P, N], BF16, kind="ExternalOutput").ap()

    with tile.TileContext(nc) as tc:
        with (
            tc.tile_pool(name="xres", bufs=1) as xpool,
            tc.tile_pool(name="wstream", bufs=6) as wpool,
            tc.tile_pool(name="outstage", bufs=8) as opool,
            tc.tile_pool(name="acc", bufs=8, space="PSUM") as ppool,
        ):
            # x^T resident: hi and lo tiles per double-k-tile, [128 kp, 2, MT, 128 m].
            # Loaded interleaved with the first n-chunk's w stream on a separate
            # DMA queue so the PE starts after one x slice + one w tile.
            xtiles = [None] * DKT
            xltiles = [None] * DKT

            def evict(nch, mt, ps):
                ot = opool.tile([P, NCH], BF16, tag="o", name=f"o{nch}_{mt}")
                if mt % 2 == 0:
                    nc.vector.tensor_copy(ot[:], ps[:])
                else:
                    nc.scalar.copy(ot[:], ps[:])
                dma_eng = nc.scalar if mt % 2 == 0 else nc.sync
                dma_eng.dma_start(out=y[mt, :, bass.ts(nch, NCH)], in_=ot[:])

            def load_x(j):
                xtile = xpool.tile([P, 2, MT, P], F8, tag=f"x{j}", name=f"x{j}")
                nc.scalar.dma_start(out=xtile[:], in_=xt[:, j])
                xtiles[j] = xtile
                if j < G:
                    xltile = xpool.tile([P, 2, MT, P], F8, tag=f"xl{j}", name=f"xl{j}")
                    nc.scalar.dma_start(out=xltile[:], in_=xl[:, j])
                    xltiles[j] = xltile

            def mms(psums, wt, jj, j, mt_range):
                last_j = j == DKT - 1
                for mt in mt_range:
                    nc.tensor.matmul(
                        out=psums[mt][:],
                        lhsT=xtiles[j][:, :, mt, :],
                        rhs=wt[:, jj],
                        start=(j == 0),
                        stop=(last_j and G < DKT),
                        perf_mode=DR,
                    )
                if j < G:
                    for mt in mt_range:
                        nc.tensor.matmul(
                            out=psums[mt][:],
                            lhsT=xltiles[j][:, :, mt, :],
                            rhs=wt[:, jj],
                            start=False,
                            stop=last_j,
                            perf_mode=DR,
                        )

            for nch in range(NCHUNKS - 1):
                psums = [
                    ppool.tile([P, NCH], F32, tag="acc", name=f"ps{nch}_{mt}")
                    for mt in range(MT)
                ]
                for wg in range(NWG):
                    wt = wpool.tile(
                        [P, WG, 2, NCH], F8, tag="w", name=f"w{nch}_{wg}"
                    )
                    nc.sync.dma_start(out=wt[:], in_=w[nch, wg])
                    for jj in range(WG):
                        j = wg * WG + jj
                        if nch == 0:
                            load_x(j)
                        mms(psums, wt, jj, j, range(MT))
                for mt in range(MT):
                    evict(nch, mt, psums[mt])

            # Last n-chunk: mt-outer / k-inner so each m-tile's accumulation
            # finishes early and its eviction + output DMA overlap the
            # remaining matmul stream; only the last m-tile drains after the
            # final matmul.
            nch = NCHUNKS - 1
            wlast = []
            for wg in range(NWG):
                wt = wpool.tile(
                    [P, WG, 2, NCH], F8, tag=f"wl{wg}", name=f"wl{wg}", bufs=1
                )
                nc.sync.dma_start(out=wt[:], in_=w[nch, wg])
                wlast.append(wt)
            for mt in range(MT):
                ps = ppool.tile([P, NCH], F32, tag="acc", name=f"psL_{mt}")
                for wg in range(NWG):
                    for jj in range(WG):
                        mms([ps] * MT, wlast[wg], jj, wg * WG + jj, [mt])
                evict(nch, mt, ps)
    nc.compile()
    return nc


def _get_program():
    global _PROGRAM
    if _PROGRAM is None:
        _PROGRAM = _build_program()
    return _PROGRAM


def _prepare_in_maps(x: np.ndarray, w: np.ndarray):
    x = np.ascontiguousarray(x, dtype=np.float32).reshape(R, K)
    w = np.ascontiguousarray(w, dtype=np.float32)
    x_hi = x.astype(E4M3)
    x_lo = (x - x_hi.astype(np.float32)).astype(E4M3)
    # rows -> [core, mt, mp, dkt, sub, kp] -> [core, kp, dkt, sub, mt, mp]
    def pack_x(a):
        ar = a.reshape(NCORES, MT, P, DKT, 2, P)
        return np.ascontiguousarray(ar.transpose(0, 5, 3, 4, 1, 2))

    xt_all = pack_x(x_hi)
    xl_all = pack_x(x_lo)[:, :, :G]
    # w [dkt, sub, kp, nch, nn] -> [nch, wg, kp, jj, sub, nn]
    wr = np.ascontiguousarray(
        w.reshape(NWG, WG, 2, P, NCHUNKS, NCH)
        .transpose(4, 0, 3, 1, 2, 5)
        .astype(E4M3)
    )
    return [
        {"xt": xt_all[c], "xl": np.ascontiguousarray(xl_all[c]), "w": wr}
        for c in range(NCORES)
    ]


def _gather_output(results):
    y = np.stack([np.asarray(r["y"]) for r in results])  # [core, MT, P, N] bf16
    return y.astype(np.float32).reshape(B, M, N)


def run(x: np.ndarray, w: np.ndarray, trace: bool = False):
    """Returns (y, BassKernelResults)."""
    nc = _get_program()
    in_maps = _prepare_in_maps(x, w)
    res = run_bass_kernel_spmd(
        nc, in_maps, core_ids=list(range(NCORES)), trace=trace
    )
    return _gather_output(res.results), res


def kernel(x: np.ndarray, w: np.ndarray) -> np.ndarray:
    y, _ = run(x, w, trace=False)
    return y


# revision 3
# speedup vs baseline: 1.2159x; 1.2091x over previous
"""Trainium2 Bass kernel for TernaryLinear: y[b,m,n] = sum_k x[b,m,k] * w[k,n].

Shapes: x (4, 2048, 4096) fp32, w (4096, 4096) ternary fp32 -> y (4, 2048, 4096).

Strategy: flatten x to 8192 rows, row-shard across 8 NeuronCores (1024 rows
each), replicate w. All matmuls run in fp8e4 (e4m3, bias 8) with the
MatmulPerfMode.DoubleRow perf mode: each matmul instruction contracts K=256
(two 128-row k-subtiles packed in the middle AP dim) at 0.5 PE cycles per
output column -- 4x the fp16 rate. The ternary weight {-1,0,1} is exact in
fp8. x is split into x_hi = e4m3(x) plus a residual x_lo = e4m3(x - x_hi);
both streams multiply the SAME w tiles and accumulate into the same PSUM
group, so the correction costs G/16 extra matmuls and no extra w traffic.
With G=16 the end-to-end rel err is ~1.8e-3 (bf16 output); pure fp8 would be
2.7e-2.

Per core: x^T hi/lo resident in SBUF (2 KB/partition per double-k-tile),
w streamed as [128, 2, 512] fp8 tiles, 8 PSUM banks (one per 128-row m-tile),
PSUM evicted as bf16 (host casts back to fp32). No cross-core communication.
"""

import sys

for _p in ("/opt/trn_rl_repo", "/opt/pypackages"):
    if _p not in sys.path:
        sys.path.append(_p)

import ml_dtypes
import numpy as np

import concourse.bass as bass
import concourse.bacc as bacc
import concourse.mybir as mybir
import concourse.tile as tile
from concourse.bass_utils import run_bass_kernel_spmd

P = 128
NCORES = 8
B, M, K, N = 4, 2048, 4096, 4096
R = B * M            # 8192 rows total
MR = R // NCORES     # 1024 rows per core
DKT = K // (2 * P)   # 16 double-k-tiles (256 contraction each)
MT = MR // P         # 8 m-tiles per core
NCH = 512            # moving free dim per matmul (one PSUM bank of fp32)
NCHUNKS = N // NCH   # 8
WG = 2               # double-k-tiles per w DMA
NWG = DKT // WG      # 8 w DMAs per n-chunk
G = 10               # double-k-tiles that get the x_lo residual correction
F32 = mybir.dt.float32
BF16 = mybir.dt.bfloat16
F8 = mybir.dt.float8e4
E4M3 = ml_dtypes.float8_e4m3
DR = mybir.MatmulPerfMode.DoubleRow

_PROGRAM = None


def _build_program():
    nc = bacc.Bacc(
        "TRN2",
        target_bir_lowering=False,
        debug=False,
        num_devices=NCORES,
    )
    xt = nc.dram_tensor("xt", [P, DKT, 2, MT, P], F8, kind="ExternalInput").ap()
    xl = nc.dram_tensor("xl", [P, G, 2, MT, P], F8, kind="ExternalInput").ap()
    w = nc.dram_tensor("w", [NCHUNKS, NWG, P, WG, 2, NCH], F8, kind="ExternalInput").ap()
    y = nc.dram_tensor("y", [MT, P, N], BF16, kind="ExternalOutput").ap()

    with tile.TileContext(nc) as tc:
        with (
            tc.tile_pool(name="xres", bufs=1) as xpool,
            tc.tile_pool(name="wstream", bufs=6) as wpool,
            tc.tile_pool(name="outstage", bufs=8) as opool,
            tc.tile_pool(name="acc", bufs=8, space="PSUM") as ppool,
        ):
            # x^T resident: hi and lo tiles per double-k-tile, [128 kp, 2, MT, 128 m].
            # Loaded interleaved with the first n-chunk's w stream on a separate
            # DMA queue so the PE starts after one x slice + one w tile.
            xtiles = [None] * DKT
            xltiles = [None] * DKT

            def evict(nch, mt, ps):
                ot = opool.tile([P, NCH], BF16, tag="o", name=f"o{nch}_{mt}")
                if mt % 2 == 0:
                    nc.vector.tensor_copy(ot[:], ps[:])
                else:
                    nc.scalar.copy(ot[:], ps[:])
                dma_eng = nc.scalar if mt % 2 == 0 else nc.sync
                dma_eng.dma_start(out=y[mt, :, bass.ts(nch, NCH)], in_=ot[:])

            def load_x(j):
                xtile = xpool.tile([P, 2, MT, P], F8, tag=f"x{j}", name=f"x{j}")
                nc.scalar.dma_start(out=xtile[:], in_=xt[:, j])
                xtiles[j] = xtile
                if j < G:
                    xltile = xpool.tile([P, 2, MT, P], F8, tag=f"xl{j}", name=f"xl{j}")
                    nc.scalar.dma_start(out=xltile[:], in_=xl[:, j])
                    xltiles[j] = xltile

            def mms(psums, wt, jj, j, mt_range):
                last_j = j == DKT - 1
                for mt in mt_range:
                    nc.tensor.matmul(
                        out=psums[mt][:],
                        lhsT=xtiles[j][:, :, mt, :],
                        rhs=wt[:, jj],
                        start=(j == 0),
                        stop=(last_j and G < DKT),
                        perf_mode=DR,
                    )
                if j < G:
                    for mt in mt_range:
                        nc.tensor.matmul(
                            out=psums[mt][:],
                            lhsT=xltiles[j][:, :, mt, :],
                            rhs=wt[:, jj],
                            start=False,
                            stop=last_j,
                            perf_mode=DR,
                        )

            for nch in range(NCHUNKS - 1):
                psums = [
                    ppool.tile([P, NCH], F32, tag="acc", name=f"ps{nch}_{mt}")
                    for mt in range(MT)
                ]
                for wg in range(NWG):
                    wt = wpool.tile(
                        [P, WG, 2, NCH], F8, tag="w", name=f"w{nch}_{wg}"
                    )
                    nc.sync.dma_start(out=wt[:], in_=w[nch, wg])
                    for jj in range(WG):
                        j = wg * WG + jj
                        if nch == 0:
                            load_x(j)
                        mms(psums, wt, jj, j, range(MT))
                for mt in range(MT):
                    evict(nch, mt, psums[mt])

            # Last n-chunk: mt-outer / k-inner so each m-tile's accumulation
            # finishes early and its eviction + output DMA overlap the
            # remaining matmul stream; only the last m-tile drains after the
            # final matmul.
            nch = NCHUNKS - 1
            wlast = []
            for wg in range(NWG):
                wt = wpool.tile(
                    [P, WG, 2, NCH], F8, tag=f"wl{wg}", name=f"wl{wg}", bufs=1
                )
                nc.sync.dma_start(out=wt[:], in_=w[nch, wg])
                wlast.append(wt)
            for mt in range(MT):
                ps = ppool.tile([P, NCH], F32, tag="acc", name=f"psL_{mt}")
                for wg in range(NWG):
                    for jj in range(WG):
                        mms([ps] * MT, wlast[wg], jj, wg * WG + jj, [mt])
                evict(nch, mt, ps)
    nc.compile()
    return nc


def _get_program():
    global _PROGRAM
    if _PROGRAM is None:
        _PROGRAM = _build_program()
    return _PROGRAM


def _prepare_in_maps(x: np.ndarray, w: np.ndarray):
    x = np.ascontiguousarray(x, dtype=np.float32).reshape(R, K)
    w = np.ascontiguousarray(w, dtype=np.float32)
    x_hi = x.astype(E4M3)
    x_lo = (x - x_hi.astype(np.float32)).astype(E4M3)
    # rows -> [core, mt, mp, dkt, sub, kp] -> [core, kp, dkt, sub, mt, mp]
    def pack_x(a):
        ar = a.reshape(NCORES, MT, P, DKT, 2, P)
        return np.ascontiguousarray(ar.transpose(0, 5, 3, 4, 1, 2))

    xt_all = pack_x(x_hi)
    xl_all = pack_x(x_lo)[:, :, :G]
    # w [dkt, sub, kp, nch, nn] -> [nch, wg, kp, jj, sub, nn]
    wr = np.ascontiguousarray(
        w.reshape(NWG, WG, 2, P, NCHUNKS, NCH)
        .transpose(4, 0, 3, 1, 2, 5)
        .astype(E4M3)
    )
    return [
        {"xt": xt_all[c], "xl": np.ascontiguousarray(xl_all[c]), "w": wr}
        for c in range(NCORES)
    ]


def _gather_output(results):
    y = np.stack([np.asarray(r["y"]) for r in results])  # [core, MT, P, N] bf16
    return y.astype(np.float32).reshape(B, M, N)


def run(x: np.ndarray, w: np.ndarray, trace: bool = False):
    """Returns (y, BassKernelResults)."""
    nc = _get_program()
    in_maps = _prepare_in_maps(x, w)
    res = run_bass_kernel_spmd(
        nc, in_maps, core_ids=list(range(NCORES)), trace=trace
    )
    return _gather_output(res.results), res


def kernel(x: np.ndarray, w: np.ndarray) -> np.ndarray:
    y, _ = run(x, w, trace=False)
    return y


# revision 6
# speedup vs baseline: 1.2255x; 1.0079x over previous
"""Trainium2 Bass kernel for TernaryLinear: y[b,m,n] = sum_k x[b,m,k] * w[k,n].

Shapes: x (4, 2048, 4096) fp32, w (4096, 4096) ternary fp32 -> y (4, 2048, 4096).

Strategy: flatten x to 8192 rows, row-shard across 8 NeuronCores (1024 rows
each), replicate w. All matmuls run in fp8e4 (e4m3, bias 8) with the
MatmulPerfMode.DoubleRow perf mode: each matmul instruction contracts K=256
(two 128-row k-subtiles in the middle AP dim). The PE moving port sustains
one 2-byte position per cycle, so a DoubleRow matmul retires 2 fp8 k-rows
per cycle -- 2x the bf16 MAC rate (measured 216 ns per K=256/N=512 matmul;
LDWEIGHTS fully hidden). The ternary weight {-1,0,1} is exact in fp8.

x is split into x_hi = e4m3(x) plus a residual x_lo = e4m3(x - x_hi); both
streams multiply the SAME w tiles and accumulate into the same PSUM group.
Correcting G of the 16 double-k-tiles costs G/16 extra matmuls and no extra
w traffic; rel err = 2.66e-2 * sqrt(1 - G/16) (G=10 -> 1.63e-2, measured).

Per core: x^T hi/lo resident in SBUF, w streamed as per-j [128, 2, 512] fp8
tiles double-buffered across the sync+scalar DMA queues, 8 PSUM banks (one
per 128-row m-tile), PSUM evicted as bf16 on vector/scalar with the output
DMA issued from the same engine (host casts back to fp32). No cross-core
communication.
"""

import sys

for _p in ("/opt/trn_rl_repo", "/opt/pypackages"):
    if _p not in sys.path:
        sys.path.append(_p)

import ml_dtypes
import numpy as np

import concourse.bass as bass
import concourse.bacc as bacc
import concourse.mybir as mybir
import concourse.tile as tile
from concourse.bass_utils import run_bass_kernel_spmd

P = 128
NCORES = 8
B, M, K, N = 4, 2048, 4096, 4096
R = B * M            # 8192 rows total
MR = R // NCORES     # 1024 rows per core
DKT = K // (2 * P)   # 16 double-k-tiles (256 contraction each)
MT = MR // P         # 8 m-tiles per core
NCH = 512            # moving free dim per matmul (one PSUM bank of fp32)
NCHUNKS = N // NCH   # 8
G = 10               # double-k-tiles that get the x_lo residual correction
XG = 2               # double-k-tiles per x DMA / resident tile
NXT = DKT // XG      # 8 hi x tiles
NXL = G // XG        # 5 lo x tiles (G must be a multiple of XG)
WLG = 4              # double-k-tiles per w tile in the last n-chunk
F32 = mybir.dt.float32
BF16 = mybir.dt.bfloat16
F8 = mybir.dt.float8e4
E4M3 = ml_dtypes.float8_e4m3
DR = mybir.MatmulPerfMode.DoubleRow

_PROGRAM = None


def _build_program():
    nc = bacc.Bacc(
        "TRN2",
        target_bir_lowering=False,
        debug=False,
        num_devices=NCORES,
    )
    xt = nc.dram_tensor("xt", [P, NXT, XG, 2, MT, P], F8, kind="ExternalInput").ap()
    xl = nc.dram_tensor("xl", [P, NXL, XG, 2, MT, P], F8, kind="ExternalInput").ap()
    w = nc.dram_tensor("w", [NCHUNKS, DKT, P, 2, NCH], F8, kind="ExternalInput").ap()
    y = nc.dram_tensor("y", [MT, P, N], BF16, kind="ExternalOutput").ap()

    with tile.TileContext(nc) as tc:
        with (
            tc.tile_pool(name="xres", bufs=1) as xpool,
            tc.tile_pool(name="wstream", bufs=12) as wpool,
            tc.tile_pool(name="outstage", bufs=8) as opool,
            tc.tile_pool(name="acc", bufs=8, space="PSUM") as ppool,
        ):
            # x^T resident: hi and lo tiles, [128 kp, XG, 2, MT, 128 m] each,
            # loaded on the scalar queue while the sync queue streams chunk 0's
            # w tiles, so the PE starts after one x tile + one w tile.
            xtiles = [None] * NXT
            xltiles = [None] * NXL

            def evict(nch, mt, ps):
                ot = opool.tile([P, NCH], BF16, tag="o", name=f"o{nch}_{mt}")
                if mt % 2 == 0:
                    nc.vector.tensor_copy(ot[:], ps[:])
                    dma_eng = nc.gpsimd
                else:
                    nc.scalar.copy(ot[:], ps[:])
                    dma_eng = nc.scalar
                dma_eng.dma_start(out=y[mt, :, bass.ts(nch, NCH)], in_=ot[:])

            def load_x(xg):
                xtile = xpool.tile([P, XG, 2, MT, P], F8, tag=f"x{xg}", name=f"x{xg}")
                nc.scalar.dma_start(out=xtile[:], in_=xt[:, xg])
                xtiles[xg] = xtile
                if xg < NXL:
                    xltile = xpool.tile(
                        [P, XG, 2, MT, P], F8, tag=f"xl{xg}", name=f"xl{xg}"
                    )
                    nc.scalar.dma_start(out=xltile[:], in_=xl[:, xg])
                    xltiles[xg] = xltile

            def mms(psums, wt_j, j, mt_range):
                xg, jj = divmod(j, XG)
                last_j = j == DKT - 1
                for mt in mt_range:
                    nc.tensor.matmul(
                        out=psums[mt][:],
                        lhsT=xtiles[xg][:, jj, :, mt, :],
                        rhs=wt_j,
                        start=(j == 0),
                        stop=(last_j and G < DKT),
                        perf_mode=DR,
                    )
                if j < G:
                    for mt in mt_range:
                        nc.tensor.matmul(
                            out=psums[mt][:],
                            lhsT=xltiles[xg][:, jj, :, mt, :],
                            rhs=wt_j,
                            start=False,
                            stop=last_j,
                            perf_mode=DR,
                        )

            for nch in range(NCHUNKS - 1):
                psums = [
                    ppool.tile([P, NCH], F32, tag="acc", name=f"ps{nch}_{mt}")
                    for mt in range(MT)
                ]
                for j in range(DKT):
                    wt = wpool.tile([P, 2, NCH], F8, tag="w", name=f"w{nch}_{j}")
                    # chunk 0: scalar queue is busy with the x preload, keep w
                    # on sync; afterwards alternate the two queues.
                    weng = nc.sync if (nch == 0 or j % 2 == 0) else nc.scalar
                    weng.dma_start(out=wt[:], in_=w[nch, j])
                    if nch == 0 and j % XG == 0:
                        load_x(j // XG)
                    mms(psums, wt[:], j, range(MT))
                for mt in range(MT):
                    evict(nch, mt, psums[mt])

            # Last n-chunk: mt-outer / k-inner so each m-tile's accumulation
            # finishes early and its eviction + output DMA overlap the
            # remaining matmul stream; only the last m-tile drains after the
            # final matmul. Its w tiles are pinned (all 16 j live at once).
            nch = NCHUNKS - 1
            wlast = []
            for wg in range(DKT // WLG):
                wt = wpool.tile(
                    [P, WLG, 2, NCH], F8, tag=f"wl{wg}", name=f"wl{wg}", bufs=1
                )
                for i in range(WLG):
                    (nc.sync if (wg * WLG + i) % 2 == 0 else nc.scalar).dma_start(
                        out=wt[:, i], in_=w[nch, wg * WLG + i]
                    )
                wlast.append(wt)
            for mt in range(MT):
                ps = ppool.tile([P, NCH], F32, tag="acc", name=f"psL_{mt}")
                for j in range(DKT):
                    mms([ps] * MT, wlast[j // WLG][:, j % WLG], j, [mt])
                evict(nch, mt, ps)
    nc.compile()
    return nc


def _get_program():
    global _PROGRAM
    if _PROGRAM is None:
        _PROGRAM = _build_program()
    return _PROGRAM


def _prepare_in_maps(x: np.ndarray, w: np.ndarray):
    x = np.ascontiguousarray(x, dtype=np.float32).reshape(R, K)
    w = np.ascontiguousarray(w, dtype=np.float32)
    x_hi = x.astype(E4M3)
    x_lo = (x - x_hi.astype(np.float32)).astype(E4M3)

    # rows -> [core, mt, mp, xg, jj, sub, kp] -> [core, kp, xg, jj, sub, mt, mp]
    def pack_x(a, ntiles):
        ar = a.reshape(NCORES, MT, P, NXT, XG, 2, P)
        return np.ascontiguousarray(ar.transpose(0, 6, 3, 4, 5, 1, 2)[:, :, :ntiles])

    xt_all = pack_x(x_hi, NXT)
    xl_all = pack_x(x_lo, NXL)
    # w [j, sub, kp, nch, nn] -> [nch, j, kp, sub, nn]
    wr = np.ascontiguousarray(
        w.reshape(DKT, 2, P, NCHUNKS, NCH).transpose(3, 0, 2, 1, 4).astype(E4M3)
    )
    return [
        {"xt": xt_all[c], "xl": xl_all[c], "w": wr}
        for c in range(NCORES)
    ]


def _gather_output(results):
    y = np.stack([np.asarray(r["y"]) for r in results])  # [core, MT, P, N] bf16
    return y.astype(np.float32).reshape(B, M, N)


def run(x: np.ndarray, w: np.ndarray, trace: bool = False):
    """Returns (y, BassKernelResults)."""
    nc = _get_program()
    in_maps = _prepare_in_maps(x, w)
    res = run_bass_kernel_spmd(
        nc, in_maps, core_ids=list(range(NCORES)), trace=trace
    )
    return _gather_output(res.results), res


def kernel(x: np.ndarray, w: np.ndarray) -> np.ndarray:
    y, _ = run(x, w, trace=False)
    return y
